# revision 1
# baseline (speedup 1.0000x reference)
"""Trainium2 Bass kernel for the 12-layer residual MLP (nn_Net_40321152975542).

Network (per row of x [B=2097152, 4]):
    h = relu(x @ W1.T + b1)                       # fc1: 4 -> 16
    res = h
    5x: h = relu(h @ Wa.T + ba)                   # A-layer 16 -> 16
        h = relu(h @ Wb.T + bb + res); res = h    # B-layer 16 -> 16 (+residual)
    y = h @ Wo.T + bo                             # head: 16 -> 2

Mapping: pure data-parallel across 8 NeuronCores (batch split).  On each core,
8 batch groups x 16 features are packed across the 128 SBUF partitions and 512
batch elements along the free dim (one "macro tile" = 4096 rows).  Every layer
is one 128x128 block-diagonal fp16 matmul (12 matmuls per macro — the PE
floor).  Residual adds do NOT use PE identity matmuls: the B-layer evacuation
is a scalar_tensor_tensor pass (t = psum + bias + res) on Pool/DVE followed by
a cheap all-fp16 relu pass, with passes balanced across Act/Pool/DVE.  Head
outputs of 8 consecutive macros are packed into one shared PSUM bank (each
macro's head matmul writes a 16-partition slice) so head evacuation + y DMA
run once per 8 macros.  fp16 (not bf16) is used end-to-end: same PE cost,
~7x lower quantization error (rel err ~1.5e-3 vs 1.0e-2).
"""

import os
import sys

sys.path.insert(0, "/opt/trn_rl_repo")

import numpy as np
import ml_dtypes
from contextlib import ExitStack

from concourse import bass, bacc, tile, mybir
from concourse.bass_utils import run_bass_kernel_spmd

F16 = np.float16

B = 2097152
N_CORES = 8
R = B // N_CORES          # rows per core
N = 512                   # free-dim columns per macro tile
G = 8                     # batch groups packed along partitions
H = 16                    # hidden width
MACRO = G * N             # rows per macro tile (4096)
N_MACROS = R // MACRO     # 64
HW = 8                    # head window: macros per shared head-psum bank

# Engine assignment tables.  Keys: layer k (0=fc1, 1..10 hidden, odd=A even=B).
# Value: engine char or tuple cycled by macro index.  'A'=Act 'P'=Pool 'V'=DVE.
# Engine assignment, tuned to MEASURED hw per-pass costs (ns, [128,512]):
#   Act relu+bias from PSUM: 723     DVE tensor_scalar relu+bias: 425
#   DVE stt (psum+bias+res): 800     DVE relu2 (sbuf fp16): 561
#   Pool: cannot access PSUM, ~7us for sbuf passes -> unused.
# Residual adds ride on the PE as fp16 identity-matmul accumulates (213 ns)
# since the 2-engine evacuation wall (Act+DVE) is the binding constraint.
EVAC1 = {0: "A", 3: "A", 5: "A", 8: "A", 9: "A",
         1: "V", 2: "V", 4: "V", 6: "V", 7: "V", 10: "V"}
RELU2 = {2: "V", 4: "V", 6: "V", 8: "V", 10: "V"}
HEADE = "A"
# per-B-layer residual mode: 'pe' = identity matmul accumulate, 'stt' =
# scalar_tensor_tensor + relu2 engine passes.  Layer 10 runs as stt: its
# identity matmul collided with the head-matmul window on the PE, and DVE
# (which carries 6 of the 11 PSUM evacuations) has the slack for it.
RESID = {2: "pe", 4: "pe", 6: "pe", 8: "pe", 10: "stt"}

BEST = dict(
    ncols=512,
    n_macros=N_MACROS,
    skew=3,
    p_bufs=6,
    hp_bufs=2,
    h_bufs=20,
    t_bufs=4,
    x_bufs=8,
    y_bufs=2,
    evac1=EVAC1,
    relu2=RELU2,
    heade=HEADE,
    resid=RESID,
    xlead=8,
)

_DT_F16 = mybir.dt.float16
_DT_F32 = mybir.dt.float32


def _prep_weights(W1, b1, Wh, bh, Wo, bo):
    """Block-diagonal fp16 stationaries + replicated fp32 bias vectors."""
    w1s = np.zeros((32, 128), dtype=F16)
    for g in range(G):
        w1s[4 * g : 4 * g + 4, H * g : H * g + H] = W1.T.astype(F16)

    # K-major concat [128, 10*128]: layer l stationary = [:, 128l:128l+128]
    wab = np.zeros((128, 10 * 128), dtype=F16)
    for l in range(10):
        for g in range(G):
            wab[H * g : H * g + H, 128 * l + H * g : 128 * l + H * g + H] = (
                Wh[l].T.astype(F16)
            )

    # head stationaries [128, 2*32]: PE output tiles are 32-partition
    # aligned, so two consecutive macros share one 32-partition slot of the
    # shared head bank via PSUM accumulation.  Variant v = macro % 2 places
    # its 16 outputs (j = 16v + 2g + o) in columns [32v, 32v+32), zeros
    # elsewhere; macro t writes psum partitions [32*((t%8)//2), +32).
    wos = np.zeros((128, 2 * 32), dtype=F16)
    for v in range(2):
        for g in range(G):
            for o in range(2):
                wos[H * g : H * g + H, 32 * v + 16 * v + 2 * g + o] = Wo[o, :].astype(F16)

    # bias vectors [128, 11]: column k = bias of relu layer k (fc1, then 1..10)
    bvecs = np.zeros((128, 11), dtype=np.float32)
    bvecs[:, 0] = np.tile(b1, G)
    for l in range(10):
        bvecs[:, 1 + l] = np.tile(bh[l], G)

    ids = np.eye(128, dtype=np.float32).astype(F16)

    return w1s, wab, wos, bvecs, ids


def _prep_x(x, ncols=N, xpair=1):
    """x [B,4] f32 -> per-core [n_macros*32/xpair, ncols*xpair] fp16.

    xpair=2 packs two consecutive macros side by side in the free dim so one
    DMA loads both (halves SP descriptor-generation work)."""
    xr = x.reshape(N_CORES, -1, G, ncols, 4).transpose(0, 1, 2, 4, 3)
    xr = np.ascontiguousarray(xr).astype(F16)   # [c, t, g, 4, ncols]
    if xpair == 1:
        return xr.reshape(N_CORES, -1, ncols)
    nt = xr.shape[1]
    xr = xr.reshape(N_CORES, nt // xpair, xpair, G, 4, ncols)
    xr = xr.transpose(0, 1, 3, 4, 2, 5)         # [c, tp, g, 4, pair, ncols]
    return np.ascontiguousarray(xr).reshape(N_CORES, -1, xpair * ncols)


def _post_y(yparts, bo, ncols=N):
    """per-core [n_windows*128, ncols] f16 -> y [B, 2] (+bo).

    Window w, partition 32q + 16v + 2g + o, col n = output o of batch row
    (macro 8w + 2q + v, group g, col n)."""
    nw = N_MACROS // HW
    y = np.stack(yparts).astype(np.float32)
    y = y.reshape(N_CORES, nw, 4, 2, G, 2, ncols)    # [c, w, q, v, g, o, n]
    y = y.transpose(0, 1, 2, 3, 4, 6, 5)             # [c, w, q, v, g, n, o]
    y = np.ascontiguousarray(y).reshape(B, 2)
    return y + bo[None, :].astype(np.float32)


def _pick(table, k, t):
    v = table[k]
    if isinstance(v, tuple):
        return v[t % len(v)]
    return v


def build_module(
    n_macros=N_MACROS,
    num_devices=N_CORES,
    ncols=N,
    skew=3,
    x_bufs=8,
    h_bufs=14,
    t_bufs=4,
    y_bufs=2,
    p_bufs=6,
    hp_bufs=2,
    evac1=EVAC1,
    relu2=RELU2,
    heade=HEADE,
    resid=RESID,
    xlead=6,
    pair=1,
    xpair=1,
    repeat=1,
    hw_loop=0,
):
    """Build + compile the per-core Bass module. Returns the compiled nc."""
    assert n_macros % HW == 0
    nc = bacc.Bacc(
        "TRN2", target_bir_lowering=False, debug=False, num_devices=num_devices
    )
    x_d = nc.dram_tensor("xprep", (n_macros // xpair * 32, xpair * ncols), _DT_F16, kind="ExternalInput").ap()
    y_dt = _DT_F32 if heade == "DMA" else _DT_F16
    y_d = nc.dram_tensor("yprep", (n_macros // HW * 128, ncols), y_dt, kind="ExternalOutput").ap()
    w1_d = nc.dram_tensor("w1s", (32, 128), _DT_F16, kind="ExternalInput").ap()
    wab_d = nc.dram_tensor("wab", (128, 1280), _DT_F16, kind="ExternalInput").ap()
    wo_d = nc.dram_tensor("wos", (128, 64), _DT_F16, kind="ExternalInput").ap()
    b_d = nc.dram_tensor("bvecs", (128, 11), _DT_F32, kind="ExternalInput").ap()
    id_d = nc.dram_tensor("ids", (128, 128), _DT_F16, kind="ExternalInput").ap()

    add = mybir.AluOpType.add
    amax = mybir.AluOpType.max

    with TileCtx(nc) as (tc, ctx):
        wpool = ctx.enter_context(tc.tile_pool(name="weights", bufs=1))
        w1t = wpool.tile([32, 128], _DT_F16, tag="w1t")
        nc.sync.dma_start(w1t[:], w1_d[:, :])
        waball = wpool.tile([128, 1280], _DT_F16, tag="waball")
        nc.sync.dma_start(waball[:], wab_d[:, :])
        wabt = [waball[:, 128 * l : 128 * l + 128] for l in range(10)]
        wot = wpool.tile([128, 64], _DT_F16, tag="wot")
        nc.sync.dma_start(wot[:], wo_d[:, :])
        ball = wpool.tile([128, 11], _DT_F32, tag="ball")
        nc.sync.dma_start(ball[:], b_d[:, :])
        btiles = [ball[:, k : k + 1] for k in range(11)]
        idt = wpool.tile([128, 128], _DT_F16, tag="idt")
        nc.sync.dma_start(idt[:], id_d[:, :])

        xpool = ctx.enter_context(tc.tile_pool(name="xin", bufs=x_bufs))
        hpool = ctx.enter_context(tc.tile_pool(name="h", bufs=h_bufs))
        tpool = ctx.enter_context(tc.tile_pool(name="tp", bufs=t_bufs))
        ypool = ctx.enter_context(tc.tile_pool(name="yout", bufs=y_bufs))
        ppool = ctx.enter_context(tc.tile_pool(name="psum", bufs=p_bufs, space="PSUM"))
        hppool = ctx.enter_context(tc.tile_pool(name="hpsum", bufs=hp_bufs, space="PSUM"))

        ncb = ncols // N
        CB = [slice(N * c, N * c + N) for c in range(ncb)]

        def relu_bias(eng, h, p, k):
            # h = max(p + b_k, 0)
            if eng == "A":
                nc.scalar.activation(
                    h, p, mybir.ActivationFunctionType.Relu, bias=btiles[k]
                )
            elif eng == "P":
                nc.gpsimd.tensor_scalar(h, p, btiles[k], 0.0, op0=add, op1=amax)
            else:
                nc.vector.tensor_scalar(h, p, btiles[k], 0.0, op0=add, op1=amax)

        def stt(eng, t_out, p, k, res):
            # t = (p + b_k) + res
            e = nc.gpsimd if eng == "P" else nc.vector
            e.scalar_tensor_tensor(t_out, p, btiles[k], res, op0=add, op1=add)

        def relu_plain(eng, h, t_in):
            # h = max(t, 0)
            if eng == "A":
                nc.scalar.activation(h, t_in, mybir.ActivationFunctionType.Relu)
            elif eng == "P":
                nc.gpsimd.tensor_scalar_max(h, t_in, 0.0)
            else:
                nc.vector.tensor_scalar_max(h, t_in, 0.0)

        # per-macro pipeline state + per-window (8 macros) head psum
        st = [dict() for _ in range(n_macros)]
        win = {}

        # fine stages: e=0 x-DMA; per layer k=0..10: mm=3k+1, evac1=3k+2,
        # evac2=3k+3; head mm e=34; head evac e=35; y DMA e=36.
        E_HEADMM = 34
        E_HEADEV = 35
        E_YDMA = 36
        N_STAGES = 37

        def stage_fine(t, e):
            m = st[t]
            if e == 0:
                if t % xpair != 0:
                    return
                xt = xpool.tile([32, xpair * ncols], _DT_F16, tag="x")
                tp = t // xpair
                nc.sync.dma_start(xt[:], x_d[32 * tp : 32 * tp + 32, :])
                for j in range(xpair):
                    st[t + j]["x"] = xt[:, j * ncols : (j + 1) * ncols]
                return
            if e == E_HEADMM:
                w = t // HW
                s = t % HW
                q, v = divmod(s, 2)
                if s == 0:
                    hp_new = hppool.tile([128, ncols], _DT_F32, tag="hp")
                    win[w] = hp_new
                hp = win[w]
                h = m.pop("h10")
                for c in CB:
                    nc.tensor.matmul(
                        hp[32 * q : 32 * q + 32, c],
                        wot[:, 32 * v : 32 * v + 32],
                        h[:, c],
                        start=(v == 0),
                        stop=(v == 1),
                        tile_position=(0, 32 * q),
                    )
                return
            if e == E_HEADEV:
                if t % HW != HW - 1 or heade == "DMA":
                    return
                hp = win.pop(t // HW)
                yt = ypool.tile([128, ncols], _DT_F16, tag="y")
                if heade == "A":
                    nc.scalar.copy(yt[:], hp[:])
                else:
                    nc.vector.tensor_copy(yt[:], hp[:])
                m["y"] = yt
                return
            if e == E_YDMA:
                if t % HW != HW - 1:
                    return
                w = t // HW
                if heade == "DMA":
                    hp = win.pop(w)
                    nc.sync.dma_start(y_d[128 * w : 128 * w + 128, :], hp[:])
                else:
                    yt = m.pop("y")
                    nc.sync.dma_start(y_d[128 * w : 128 * w + 128, :], yt[:])
                return
            k, ph = divmod(e - 1, 3)
            if k > 10:
                return
            is_b = k > 0 and k % 2 == 0
            on_pe = is_b and resid[k] == "pe"
            if ph == 0:  # matmul of layer k
                p = ppool.tile([128, ncols], _DT_F32, tag="p")
                if k == 0:
                    xt = m.pop("x")
                    for c in CB:
                        nc.tensor.matmul(p[:, c], w1t[:], xt[:, c], start=True, stop=True)
                elif on_pe:
                    h = m[f"h{k - 1}"]
                    res = m[f"h{k - 2}"]
                    # Alternate accumulation order per macro so paired
                    # emission runs Wb,id | id,Wb — adjacent id matmuls share
                    # one ldweights of the identity stationary.
                    for c in CB:
                        if t % 2 == 0:
                            nc.tensor.matmul(
                                p[:, c], wabt[k - 1], h[:, c], start=True, stop=False
                            )
                            nc.tensor.matmul(
                                p[:, c], idt[:], res[:, c], start=False, stop=True
                            )
                        else:
                            nc.tensor.matmul(
                                p[:, c], idt[:], res[:, c], start=True, stop=False
                            )
                            nc.tensor.matmul(
                                p[:, c], wabt[k - 1], h[:, c], start=False, stop=True
                            )
                else:
                    h = m[f"h{k - 1}"]
                    for c in CB:
                        nc.tensor.matmul(
                            p[:, c], wabt[k - 1], h[:, c], start=True, stop=True
                        )
                m["p"] = p
                return
            if ph == 1:  # evac1: relu+bias (fc1/A/B-on-pe) or stt (B-stt)
                eng = _pick(evac1, k, t)
                if not is_b or on_pe:
                    h = hpool.tile([128, ncols], _DT_F16, tag="h")
                    relu_bias(eng, h[:], m.pop("p"), k)
                    m[f"h{k}"] = h
                else:
                    tt = tpool.tile([128, ncols], _DT_F16, tag="t")
                    res = m[f"h{k - 2}"]
                    stt(eng, tt[:], m.pop("p"), k, res[:])
                    m["t"] = tt
                return
            # ph == 2: relu2 for B layers with engine residual
            if not is_b or on_pe:
                return
            eng = _pick(relu2, k, t)
            h = hpool.tile([128, ncols], _DT_F16, tag="h")
            relu_plain(eng, h[:], m.pop("t")[:])
            m[f"h{k}"] = h

        # pair>1 emits each fine stage for `pair` consecutive macros
        # back-to-back, so same-stationary matmuls run adjacently on the PE
        # (ldweights amortization).
        events = sorted(
            (((t // pair) * skew + (e if e else -xlead), t, e)
             for t in range(n_macros) for e in range(N_STAGES)),
            key=(lambda ev: (ev[0], ev[1])) if pair == 1 else
                (lambda ev: (ev[0], ev[2], ev[1])),
        )

        def emit_all():
            for rep in range(repeat):
                st[:] = [dict() for _ in range(n_macros)]
                win.clear()
                for _, t, e in events:
                    stage_fine(t, e)

        if hw_loop:
            with tc.For_i(0, hw_loop, 1):
                emit_all()
        else:
            emit_all()

    nc.compile()
    return nc


class TileCtx:
    """TileContext + ExitStack in one `with`."""

    def __init__(self, nc):
        self.nc = nc

    def __enter__(self):
        self._es = ExitStack()
        self._tc = self._es.enter_context(tile.TileContext(self.nc))
        return self._tc, self._es

    def __exit__(self, *exc):
        return self._es.__exit__(*exc)


_CACHED_NC = None


def kernel(x, W1, b1, Wh, bh, Wo, bo):
    global _CACHED_NC
    x = np.asarray(x, dtype=np.float32)
    W1 = np.asarray(W1, dtype=np.float32)
    b1 = np.asarray(b1, dtype=np.float32)
    Wh = np.asarray(Wh, dtype=np.float32)
    bh = np.asarray(bh, dtype=np.float32)
    Wo = np.asarray(Wo, dtype=np.float32)
    bo = np.asarray(bo, dtype=np.float32)

    w1s, wab, wos, bvecs, ids = _prep_weights(W1, b1, Wh, bh, Wo, bo)
    xprep = _prep_x(x, ncols=BEST["ncols"], xpair=BEST.get("xpair", 1))

    if _CACHED_NC is None:
        _CACHED_NC = build_module(num_devices=N_CORES, **BEST)
    nc = _CACHED_NC

    in_maps = [
        {
            "xprep": np.ascontiguousarray(xprep[c]),
            "w1s": w1s,
            "wab": wab,
            "wos": wos,
            "bvecs": bvecs,
            "ids": ids,
        }
        for c in range(N_CORES)
    ]
    res = run_bass_kernel_spmd(nc, in_maps, core_ids=list(range(N_CORES)))
    yparts = [res.results[c]["yprep"] for c in range(N_CORES)]
    return _post_y(yparts, bo, ncols=BEST["ncols"])



# revision 2
# speedup vs baseline: 1.2202x; 1.2202x over previous
"""Trainium2 Bass kernel for the 12-layer residual MLP (nn_Net_40321152975542).

Network (per row of x [B=2097152, 4]):
    h = relu(x @ W1.T + b1)                       # fc1: 4 -> 16
    res = h
    5x: h = relu(h @ Wa.T + ba)                   # A-layer 16 -> 16
        h = relu(h @ Wb.T + bb + res); res = h    # B-layer 16 -> 16 (+residual)
    y = h @ Wo.T + bo                             # head: 16 -> 2

Mapping: pure data-parallel across 8 NeuronCores (batch split).  On each core,
8 batch groups x 16 features are packed across the 128 SBUF partitions and 512
batch elements along the free dim (one "macro tile" = 4096 rows).  Every layer
is one 128x128 block-diagonal fp16 matmul (12 matmuls per macro — the PE
floor).  Residual adds do NOT use PE identity matmuls: the B-layer evacuation
is a scalar_tensor_tensor pass (t = psum + bias + res) on Pool/DVE followed by
a cheap all-fp16 relu pass, with passes balanced across Act/Pool/DVE.  Head
outputs of 8 consecutive macros are packed into one shared PSUM bank (each
macro's head matmul writes a 16-partition slice) so head evacuation + y DMA
run once per 8 macros.  fp16 (not bf16) is used end-to-end: same PE cost,
~7x lower quantization error (rel err ~1.5e-3 vs 1.0e-2).
"""

import os
import sys

sys.path.insert(0, "/opt/trn_rl_repo")

import numpy as np
import ml_dtypes
from contextlib import ExitStack

from concourse import bass, bacc, tile, mybir
from concourse.bass_utils import run_bass_kernel_spmd

F16 = np.float16

B = 2097152
N_CORES = 8
R = B // N_CORES          # rows per core
N = 512                   # free-dim columns per macro tile
G = 8                     # batch groups packed along partitions
H = 16                    # hidden width
MACRO = G * N             # rows per macro tile (4096)
N_MACROS = R // MACRO     # 64
HW = 8                    # head window: macros per shared head-psum bank

# Engine assignment tables.  Keys: layer k (0=fc1, 1..10 hidden, odd=A even=B).
# Value: engine char or tuple cycled by macro index.  'A'=Act 'P'=Pool 'V'=DVE.
# Engine assignment, tuned to MEASURED hw per-pass costs (ns, [128,512]):
#   Act relu+bias from PSUM: 723     DVE tensor_scalar relu+bias: 425
#   DVE stt (psum+bias+res): 800     DVE relu2 (sbuf fp16): 561
#   Pool: cannot access PSUM, ~7us for sbuf passes -> unused.
# Residual adds ride on the PE as fp16 identity-matmul accumulates (213 ns)
# since the 2-engine evacuation wall (Act+DVE) is the binding constraint.
EVAC1 = {0: "A", 3: "A", 5: "A", 8: "A", 9: "A", 10: "A",
         1: "V", 2: "V", 4: "V", 6: "V", 7: "V"}
RELU2 = {2: "V", 4: "V", 6: "V", 8: "V", 10: "V"}
HEADE = "A"
# per-B-layer residual mode: 'pe' = identity matmul accumulate, 'stt' =
# scalar_tensor_tensor + relu2 engine passes.  All five run as 'pe': DVE is
# the binding engine, so the k=10 stt+relu2 pair costs more on DVE than the
# extra identity matmul costs on the PE (HW-swept: 270us vs 276us).
RESID = {2: "pe", 4: "pe", 6: "pe", 8: "pe", 10: "pe"}

BEST = dict(
    ncols=512,
    n_macros=N_MACROS,
    skew=3,
    p_bufs=6,
    hp_bufs=2,
    h_bufs=20,
    t_bufs=4,
    x_bufs=8,
    y_bufs=2,
    evac1=EVAC1,
    relu2=RELU2,
    heade=HEADE,
    resid=RESID,
    xlead=8,
)

_DT_F16 = mybir.dt.float16
_DT_F32 = mybir.dt.float32


def _prep_weights(W1, b1, Wh, bh, Wo, bo):
    """Block-diagonal fp16 stationaries + replicated fp32 bias vectors."""
    w1s = np.zeros((32, 128), dtype=F16)
    for g in range(G):
        w1s[4 * g : 4 * g + 4, H * g : H * g + H] = W1.T.astype(F16)

    # K-major concat [128, 10*128]: layer l stationary = [:, 128l:128l+128]
    wab = np.zeros((128, 10 * 128), dtype=F16)
    for l in range(10):
        for g in range(G):
            wab[H * g : H * g + H, 128 * l + H * g : 128 * l + H * g + H] = (
                Wh[l].T.astype(F16)
            )

    # head stationaries [128, 2*32]: PE output tiles are 32-partition
    # aligned, so two consecutive macros share one 32-partition slot of the
    # shared head bank via PSUM accumulation.  Variant v = macro % 2 places
    # its 16 outputs (j = 16v + 2g + o) in columns [32v, 32v+32), zeros
    # elsewhere; macro t writes psum partitions [32*((t%8)//2), +32).
    wos = np.zeros((128, 2 * 32), dtype=F16)
    for v in range(2):
        for g in range(G):
            for o in range(2):
                wos[H * g : H * g + H, 32 * v + 16 * v + 2 * g + o] = Wo[o, :].astype(F16)

    # bias vectors [128, 11]: column k = bias of relu layer k (fc1, then 1..10)
    bvecs = np.zeros((128, 11), dtype=np.float32)
    bvecs[:, 0] = np.tile(b1, G)
    for l in range(10):
        bvecs[:, 1 + l] = np.tile(bh[l], G)

    ids = np.eye(128, dtype=np.float32).astype(F16)

    return w1s, wab, wos, bvecs, ids


def _prep_x(x, ncols=N, xpair=1):
    """x [B,4] f32 -> per-core [n_macros*32/xpair, ncols*xpair] fp16.

    xpair=2 packs two consecutive macros side by side in the free dim so one
    DMA loads both (halves SP descriptor-generation work)."""
    xr = x.reshape(N_CORES, -1, G, ncols, 4).transpose(0, 1, 2, 4, 3)
    xr = np.ascontiguousarray(xr).astype(F16)   # [c, t, g, 4, ncols]
    if xpair == 1:
        return xr.reshape(N_CORES, -1, ncols)
    nt = xr.shape[1]
    xr = xr.reshape(N_CORES, nt // xpair, xpair, G, 4, ncols)
    xr = xr.transpose(0, 1, 3, 4, 2, 5)         # [c, tp, g, 4, pair, ncols]
    return np.ascontiguousarray(xr).reshape(N_CORES, -1, xpair * ncols)


def _post_y(yparts, bo, ncols=N):
    """per-core [n_windows*128, ncols] f16 -> y [B, 2] (+bo).

    Window w, partition 32q + 16v + 2g + o, col n = output o of batch row
    (macro 8w + 2q + v, group g, col n)."""
    nw = N_MACROS // HW
    y = np.stack(yparts).astype(np.float32)
    y = y.reshape(N_CORES, nw, 4, 2, G, 2, ncols)    # [c, w, q, v, g, o, n]
    y = y.transpose(0, 1, 2, 3, 4, 6, 5)             # [c, w, q, v, g, n, o]
    y = np.ascontiguousarray(y).reshape(B, 2)
    return y + bo[None, :].astype(np.float32)


def _pick(table, k, t):
    v = table[k]
    if isinstance(v, tuple):
        return v[t % len(v)]
    return v


def build_module(
    n_macros=N_MACROS,
    num_devices=N_CORES,
    ncols=N,
    skew=3,
    x_bufs=8,
    h_bufs=14,
    t_bufs=4,
    y_bufs=2,
    p_bufs=6,
    hp_bufs=2,
    evac1=EVAC1,
    relu2=RELU2,
    heade=HEADE,
    resid=RESID,
    xlead=6,
    pair=1,
    xpair=1,
    repeat=1,
    hw_loop=0,
):
    """Build + compile the per-core Bass module. Returns the compiled nc."""
    assert n_macros % HW == 0
    nc = bacc.Bacc(
        "TRN2", target_bir_lowering=False, debug=False, num_devices=num_devices
    )
    x_d = nc.dram_tensor("xprep", (n_macros // xpair * 32, xpair * ncols), _DT_F16, kind="ExternalInput").ap()
    y_dt = _DT_F32 if heade == "DMA" else _DT_F16
    y_d = nc.dram_tensor("yprep", (n_macros // HW * 128, ncols), y_dt, kind="ExternalOutput").ap()
    w1_d = nc.dram_tensor("w1s", (32, 128), _DT_F16, kind="ExternalInput").ap()
    wab_d = nc.dram_tensor("wab", (128, 1280), _DT_F16, kind="ExternalInput").ap()
    wo_d = nc.dram_tensor("wos", (128, 64), _DT_F16, kind="ExternalInput").ap()
    b_d = nc.dram_tensor("bvecs", (128, 11), _DT_F32, kind="ExternalInput").ap()
    id_d = nc.dram_tensor("ids", (128, 128), _DT_F16, kind="ExternalInput").ap()

    add = mybir.AluOpType.add
    amax = mybir.AluOpType.max

    with TileCtx(nc) as (tc, ctx):
        wpool = ctx.enter_context(tc.tile_pool(name="weights", bufs=1))
        w1t = wpool.tile([32, 128], _DT_F16, tag="w1t")
        nc.sync.dma_start(w1t[:], w1_d[:, :])
        waball = wpool.tile([128, 1280], _DT_F16, tag="waball")
        nc.sync.dma_start(waball[:], wab_d[:, :])
        wabt = [waball[:, 128 * l : 128 * l + 128] for l in range(10)]
        wot = wpool.tile([128, 64], _DT_F16, tag="wot")
        nc.sync.dma_start(wot[:], wo_d[:, :])
        ball = wpool.tile([128, 11], _DT_F32, tag="ball")
        nc.sync.dma_start(ball[:], b_d[:, :])
        btiles = [ball[:, k : k + 1] for k in range(11)]
        idt = wpool.tile([128, 128], _DT_F16, tag="idt")
        nc.sync.dma_start(idt[:], id_d[:, :])

        xpool = ctx.enter_context(tc.tile_pool(name="xin", bufs=x_bufs))
        hpool = ctx.enter_context(tc.tile_pool(name="h", bufs=h_bufs))
        tpool = ctx.enter_context(tc.tile_pool(name="tp", bufs=t_bufs))
        ypool = ctx.enter_context(tc.tile_pool(name="yout", bufs=y_bufs))
        ppool = ctx.enter_context(tc.tile_pool(name="psum", bufs=p_bufs, space="PSUM"))
        hppool = ctx.enter_context(tc.tile_pool(name="hpsum", bufs=hp_bufs, space="PSUM"))

        ncb = ncols // N
        CB = [slice(N * c, N * c + N) for c in range(ncb)]

        def relu_bias(eng, h, p, k):
            # h = max(p + b_k, 0)
            if eng == "A":
                nc.scalar.activation(
                    h, p, mybir.ActivationFunctionType.Relu, bias=btiles[k]
                )
            elif eng == "P":
                nc.gpsimd.tensor_scalar(h, p, btiles[k], 0.0, op0=add, op1=amax)
            else:
                nc.vector.tensor_scalar(h, p, btiles[k], 0.0, op0=add, op1=amax)

        def stt(eng, t_out, p, k, res):
            # t = (p + b_k) + res
            e = nc.gpsimd if eng == "P" else nc.vector
            e.scalar_tensor_tensor(t_out, p, btiles[k], res, op0=add, op1=add)

        def relu_plain(eng, h, t_in):
            # h = max(t, 0)
            if eng == "A":
                nc.scalar.activation(h, t_in, mybir.ActivationFunctionType.Relu)
            elif eng == "P":
                nc.gpsimd.tensor_scalar_max(h, t_in, 0.0)
            else:
                nc.vector.tensor_scalar_max(h, t_in, 0.0)

        # per-macro pipeline state + per-window (8 macros) head psum
        st = [dict() for _ in range(n_macros)]
        win = {}

        # fine stages: e=0 x-DMA; per layer k=0..10: mm=3k+1, evac1=3k+2,
        # evac2=3k+3; head mm e=34; head evac e=35; y DMA e=36.
        E_HEADMM = 34
        E_HEADEV = 35
        E_YDMA = 36
        N_STAGES = 37

        def stage_fine(t, e):
            m = st[t]
            if e == 0:
                if t % xpair != 0:
                    return
                xt = xpool.tile([32, xpair * ncols], _DT_F16, tag="x")
                tp = t // xpair
                nc.sync.dma_start(xt[:], x_d[32 * tp : 32 * tp + 32, :])
                for j in range(xpair):
                    st[t + j]["x"] = xt[:, j * ncols : (j + 1) * ncols]
                return
            if e == E_HEADMM:
                w = t // HW
                s = t % HW
                q, v = divmod(s, 2)
                if s == 0:
                    hp_new = hppool.tile([128, ncols], _DT_F32, tag="hp")
                    win[w] = hp_new
                hp = win[w]
                h = m.pop("h10")
                for c in CB:
                    nc.tensor.matmul(
                        hp[32 * q : 32 * q + 32, c],
                        wot[:, 32 * v : 32 * v + 32],
                        h[:, c],
                        start=(v == 0),
                        stop=(v == 1),
                        tile_position=(0, 32 * q),
                    )
                return
            if e == E_HEADEV:
                if t % HW != HW - 1 or heade == "DMA":
                    return
                hp = win.pop(t // HW)
                yt = ypool.tile([128, ncols], _DT_F16, tag="y")
                if heade == "A":
                    nc.scalar.copy(yt[:], hp[:])
                else:
                    nc.vector.tensor_copy(yt[:], hp[:])
                m["y"] = yt
                return
            if e == E_YDMA:
                if t % HW != HW - 1:
                    return
                w = t // HW
                if heade == "DMA":
                    hp = win.pop(w)
                    nc.sync.dma_start(y_d[128 * w : 128 * w + 128, :], hp[:])
                else:
                    yt = m.pop("y")
                    nc.sync.dma_start(y_d[128 * w : 128 * w + 128, :], yt[:])
                return
            k, ph = divmod(e - 1, 3)
            if k > 10:
                return
            is_b = k > 0 and k % 2 == 0
            on_pe = is_b and resid[k] == "pe"
            if ph == 0:  # matmul of layer k
                p = ppool.tile([128, ncols], _DT_F32, tag="p")
                if k == 0:
                    xt = m.pop("x")
                    for c in CB:
                        nc.tensor.matmul(p[:, c], w1t[:], xt[:, c], start=True, stop=True)
                elif on_pe:
                    h = m[f"h{k - 1}"]
                    res = m[f"h{k - 2}"]
                    # Alternate accumulation order per macro so paired
                    # emission runs Wb,id | id,Wb — adjacent id matmuls share
                    # one ldweights of the identity stationary.
                    for c in CB:
                        if t % 2 == 0:
                            nc.tensor.matmul(
                                p[:, c], wabt[k - 1], h[:, c], start=True, stop=False
                            )
                            nc.tensor.matmul(
                                p[:, c], idt[:], res[:, c], start=False, stop=True
                            )
                        else:
                            nc.tensor.matmul(
                                p[:, c], idt[:], res[:, c], start=True, stop=False
                            )
                            nc.tensor.matmul(
                                p[:, c], wabt[k - 1], h[:, c], start=False, stop=True
                            )
                else:
                    h = m[f"h{k - 1}"]
                    for c in CB:
                        nc.tensor.matmul(
                            p[:, c], wabt[k - 1], h[:, c], start=True, stop=True
                        )
                m["p"] = p
                return
            if ph == 1:  # evac1: relu+bias (fc1/A/B-on-pe) or stt (B-stt)
                eng = _pick(evac1, k, t)
                if not is_b or on_pe:
                    h = hpool.tile([128, ncols], _DT_F16, tag="h")
                    relu_bias(eng, h[:], m.pop("p"), k)
                    m[f"h{k}"] = h
                else:
                    tt = tpool.tile([128, ncols], _DT_F16, tag="t")
                    res = m[f"h{k - 2}"]
                    stt(eng, tt[:], m.pop("p"), k, res[:])
                    m["t"] = tt
                return
            # ph == 2: relu2 for B layers with engine residual
            if not is_b or on_pe:
                return
            eng = _pick(relu2, k, t)
            h = hpool.tile([128, ncols], _DT_F16, tag="h")
            relu_plain(eng, h[:], m.pop("t")[:])
            m[f"h{k}"] = h

        # pair>1 emits each fine stage for `pair` consecutive macros
        # back-to-back, so same-stationary matmuls run adjacently on the PE
        # (ldweights amortization).
        events = sorted(
            (((t // pair) * skew + (e if e else -xlead), t, e)
             for t in range(n_macros) for e in range(N_STAGES)),
            key=(lambda ev: (ev[0], ev[1])) if pair == 1 else
                (lambda ev: (ev[0], ev[2], ev[1])),
        )

        def emit_all():
            for rep in range(repeat):
                st[:] = [dict() for _ in range(n_macros)]
                win.clear()
                for _, t, e in events:
                    stage_fine(t, e)

        if hw_loop:
            with tc.For_i(0, hw_loop, 1):
                emit_all()
        else:
            emit_all()

    nc.compile()
    return nc


class TileCtx:
    """TileContext + ExitStack in one `with`."""

    def __init__(self, nc):
        self.nc = nc

    def __enter__(self):
        self._es = ExitStack()
        self._tc = self._es.enter_context(tile.TileContext(self.nc))
        return self._tc, self._es

    def __exit__(self, *exc):
        return self._es.__exit__(*exc)


_CACHED_NC = None


def kernel(x, W1, b1, Wh, bh, Wo, bo):
    global _CACHED_NC
    x = np.asarray(x, dtype=np.float32)
    W1 = np.asarray(W1, dtype=np.float32)
    b1 = np.asarray(b1, dtype=np.float32)
    Wh = np.asarray(Wh, dtype=np.float32)
    bh = np.asarray(bh, dtype=np.float32)
    Wo = np.asarray(Wo, dtype=np.float32)
    bo = np.asarray(bo, dtype=np.float32)

    w1s, wab, wos, bvecs, ids = _prep_weights(W1, b1, Wh, bh, Wo, bo)
    xprep = _prep_x(x, ncols=BEST["ncols"], xpair=BEST.get("xpair", 1))

    if _CACHED_NC is None:
        _CACHED_NC = build_module(num_devices=N_CORES, **BEST)
    nc = _CACHED_NC

    in_maps = [
        {
            "xprep": np.ascontiguousarray(xprep[c]),
            "w1s": w1s,
            "wab": wab,
            "wos": wos,
            "bvecs": bvecs,
            "ids": ids,
        }
        for c in range(N_CORES)
    ]
    res = run_bass_kernel_spmd(nc, in_maps, core_ids=list(range(N_CORES)))
    yparts = [res.results[c]["yprep"] for c in range(N_CORES)]
    return _post_y(yparts, bo, ncols=BEST["ncols"])



# revision 3
# speedup vs baseline: 1.2310x; 1.0088x over previous
"""Trainium2 Bass kernel for the 12-layer residual MLP (nn_Net_40321152975542).

Network (per row of x [B=2097152, 4]):
    h = relu(x @ W1.T + b1)                       # fc1: 4 -> 16
    res = h
    5x: h = relu(h @ Wa.T + ba)                   # A-layer 16 -> 16
        h = relu(h @ Wb.T + bb + res); res = h    # B-layer 16 -> 16 (+residual)
    y = h @ Wo.T + bo                             # head: 16 -> 2

Mapping: pure data-parallel across 8 NeuronCores (batch split).  On each core,
8 batch groups x 16 features are packed across the 128 SBUF partitions and 512
batch elements along the free dim (one "macro tile" = 4096 rows).  Every layer
is one 128x128 block-diagonal fp16 matmul (12 matmuls per macro — the PE
floor).  Residual adds do NOT use PE identity matmuls: the B-layer evacuation
is a scalar_tensor_tensor pass (t = psum + bias + res) on Pool/DVE followed by
a cheap all-fp16 relu pass, with passes balanced across Act/Pool/DVE.  Head
outputs of 8 consecutive macros are packed into one shared PSUM bank (each
macro's head matmul writes a 16-partition slice) so head evacuation + y DMA
run once per 8 macros.  fp16 (not bf16) is used end-to-end: same PE cost,
~7x lower quantization error (rel err ~1.5e-3 vs 1.0e-2).
"""

import os
import sys

sys.path.insert(0, "/opt/trn_rl_repo")

import numpy as np
import ml_dtypes
from contextlib import ExitStack

from concourse import bass, bacc, tile, mybir
from concourse.bass_utils import run_bass_kernel_spmd

F16 = np.float16

B = 2097152
N_CORES = 8
R = B // N_CORES          # rows per core
N = 512                   # free-dim columns per macro tile
G = 8                     # batch groups packed along partitions
H = 16                    # hidden width
MACRO = G * N             # rows per macro tile (4096)
N_MACROS = R // MACRO     # 64
HW = 8                    # head window: macros per shared head-psum bank

# Engine assignment tables.  Keys: layer k (0=fc1, 1..10 hidden, odd=A even=B).
# Value: engine char or tuple cycled by macro index.  'A'=Act 'P'=Pool 'V'=DVE.
# Engine assignment, tuned to MEASURED hw per-pass costs (ns, [128,512]):
#   Act relu+bias from PSUM: 723     DVE tensor_scalar relu+bias: 425
#   DVE stt (psum+bias+res): 800     DVE relu2 (sbuf fp16): 561
#   Pool: cannot access PSUM, ~7us for sbuf passes -> unused.
# Residual adds ride on the PE as fp16 identity-matmul accumulates (213 ns)
# since the 2-engine evacuation wall (Act+DVE) is the binding constraint.
EVAC1 = {0: "A", 3: "A", 5: "A", 8: "A", 9: "A", 10: "A",
         1: "V", 2: "V", 4: "V", 6: "V", 7: "V"}
RELU2 = {2: "V", 4: "V", 6: "V", 8: "V", 10: "V"}
HEADE = "V"
# per-B-layer residual mode: 'pe' = identity matmul accumulate, 'stt' =
# scalar_tensor_tensor + relu2 engine passes.  All five run as 'pe': DVE is
# the binding engine, so the k=10 stt+relu2 pair costs more on DVE than the
# extra identity matmul costs on the PE (HW-swept: 270us vs 276us).
RESID = {2: "pe", 4: "pe", 6: "pe", 8: "pe", 10: "pe"}

BEST = dict(
    ncols=512,
    n_macros=N_MACROS,
    skew=3,
    p_bufs=6,
    hp_bufs=2,
    h_bufs=20,
    t_bufs=4,
    x_bufs=8,
    y_bufs=2,
    evac1=EVAC1,
    relu2=RELU2,
    heade=HEADE,
    resid=RESID,
    xlead=8,
)

_DT_F16 = mybir.dt.float16
_DT_F32 = mybir.dt.float32


def _prep_weights(W1, b1, Wh, bh, Wo, bo):
    """Block-diagonal fp16 stationaries + replicated fp32 bias vectors."""
    w1s = np.zeros((32, 128), dtype=F16)
    for g in range(G):
        w1s[4 * g : 4 * g + 4, H * g : H * g + H] = W1.T.astype(F16)

    # K-major concat [128, 10*128]: layer l stationary = [:, 128l:128l+128]
    wab = np.zeros((128, 10 * 128), dtype=F16)
    for l in range(10):
        for g in range(G):
            wab[H * g : H * g + H, 128 * l + H * g : 128 * l + H * g + H] = (
                Wh[l].T.astype(F16)
            )

    # head stationaries [128, 2*32]: PE output tiles are 32-partition
    # aligned, so two consecutive macros share one 32-partition slot of the
    # shared head bank via PSUM accumulation.  Variant v = macro % 2 places
    # its 16 outputs (j = 16v + 2g + o) in columns [32v, 32v+32), zeros
    # elsewhere; macro t writes psum partitions [32*((t%8)//2), +32).
    wos = np.zeros((128, 2 * 32), dtype=F16)
    for v in range(2):
        for g in range(G):
            for o in range(2):
                wos[H * g : H * g + H, 32 * v + 16 * v + 2 * g + o] = Wo[o, :].astype(F16)

    # bias vectors [128, 11]: column k = bias of relu layer k (fc1, then 1..10)
    bvecs = np.zeros((128, 11), dtype=np.float32)
    bvecs[:, 0] = np.tile(b1, G)
    for l in range(10):
        bvecs[:, 1 + l] = np.tile(bh[l], G)

    ids = np.eye(128, dtype=np.float32).astype(F16)

    return w1s, wab, wos, bvecs, ids


def _prep_x(x, ncols=N, xpair=1):
    """x [B,4] f32 -> per-core [n_macros*32/xpair, ncols*xpair] fp16.

    xpair=2 packs two consecutive macros side by side in the free dim so one
    DMA loads both (halves SP descriptor-generation work)."""
    xr = x.reshape(N_CORES, -1, G, ncols, 4).transpose(0, 1, 2, 4, 3)
    xr = np.ascontiguousarray(xr).astype(F16)   # [c, t, g, 4, ncols]
    if xpair == 1:
        return xr.reshape(N_CORES, -1, ncols)
    nt = xr.shape[1]
    xr = xr.reshape(N_CORES, nt // xpair, xpair, G, 4, ncols)
    xr = xr.transpose(0, 1, 3, 4, 2, 5)         # [c, tp, g, 4, pair, ncols]
    return np.ascontiguousarray(xr).reshape(N_CORES, -1, xpair * ncols)


def _post_y(yparts, bo, ncols=N):
    """per-core [n_windows*128, ncols] f16 -> y [B, 2] (+bo).

    Window w, partition 32q + 16v + 2g + o, col n = output o of batch row
    (macro 8w + 2q + v, group g, col n)."""
    nw = N_MACROS // HW
    y = np.stack(yparts).astype(np.float32)
    y = y.reshape(N_CORES, nw, 4, 2, G, 2, ncols)    # [c, w, q, v, g, o, n]
    y = y.transpose(0, 1, 2, 3, 4, 6, 5)             # [c, w, q, v, g, n, o]
    y = np.ascontiguousarray(y).reshape(B, 2)
    return y + bo[None, :].astype(np.float32)


def _pick(table, k, t):
    v = table[k]
    if isinstance(v, tuple):
        return v[t % len(v)]
    return v


def build_module(
    n_macros=N_MACROS,
    num_devices=N_CORES,
    ncols=N,
    skew=3,
    x_bufs=8,
    h_bufs=14,
    t_bufs=4,
    y_bufs=2,
    p_bufs=6,
    hp_bufs=2,
    evac1=EVAC1,
    relu2=RELU2,
    heade=HEADE,
    resid=RESID,
    xlead=6,
    pair=1,
    xpair=1,
    repeat=1,
    hw_loop=0,
):
    """Build + compile the per-core Bass module. Returns the compiled nc."""
    assert n_macros % HW == 0
    nc = bacc.Bacc(
        "TRN2", target_bir_lowering=False, debug=False, num_devices=num_devices
    )
    x_d = nc.dram_tensor("xprep", (n_macros // xpair * 32, xpair * ncols), _DT_F16, kind="ExternalInput").ap()
    y_dt = _DT_F32 if heade == "DMA" else _DT_F16
    y_d = nc.dram_tensor("yprep", (n_macros // HW * 128, ncols), y_dt, kind="ExternalOutput").ap()
    w1_d = nc.dram_tensor("w1s", (32, 128), _DT_F16, kind="ExternalInput").ap()
    wab_d = nc.dram_tensor("wab", (128, 1280), _DT_F16, kind="ExternalInput").ap()
    wo_d = nc.dram_tensor("wos", (128, 64), _DT_F16, kind="ExternalInput").ap()
    b_d = nc.dram_tensor("bvecs", (128, 11), _DT_F32, kind="ExternalInput").ap()
    id_d = nc.dram_tensor("ids", (128, 128), _DT_F16, kind="ExternalInput").ap()

    add = mybir.AluOpType.add
    amax = mybir.AluOpType.max

    with TileCtx(nc) as (tc, ctx):
        wpool = ctx.enter_context(tc.tile_pool(name="weights", bufs=1))
        w1t = wpool.tile([32, 128], _DT_F16, tag="w1t")
        nc.sync.dma_start(w1t[:], w1_d[:, :])
        waball = wpool.tile([128, 1280], _DT_F16, tag="waball")
        nc.sync.dma_start(waball[:], wab_d[:, :])
        wabt = [waball[:, 128 * l : 128 * l + 128] for l in range(10)]
        wot = wpool.tile([128, 64], _DT_F16, tag="wot")
        nc.sync.dma_start(wot[:], wo_d[:, :])
        ball = wpool.tile([128, 11], _DT_F32, tag="ball")
        nc.sync.dma_start(ball[:], b_d[:, :])
        btiles = [ball[:, k : k + 1] for k in range(11)]
        idt = wpool.tile([128, 128], _DT_F16, tag="idt")
        nc.sync.dma_start(idt[:], id_d[:, :])

        xpool = ctx.enter_context(tc.tile_pool(name="xin", bufs=x_bufs))
        hpool = ctx.enter_context(tc.tile_pool(name="h", bufs=h_bufs))
        tpool = ctx.enter_context(tc.tile_pool(name="tp", bufs=t_bufs))
        ypool = ctx.enter_context(tc.tile_pool(name="yout", bufs=y_bufs))
        ppool = ctx.enter_context(tc.tile_pool(name="psum", bufs=p_bufs, space="PSUM"))
        hppool = ctx.enter_context(tc.tile_pool(name="hpsum", bufs=hp_bufs, space="PSUM"))

        ncb = ncols // N
        CB = [slice(N * c, N * c + N) for c in range(ncb)]

        def relu_bias(eng, h, p, k):
            # h = max(p + b_k, 0)
            if eng == "A":
                nc.scalar.activation(
                    h, p, mybir.ActivationFunctionType.Relu, bias=btiles[k]
                )
            elif eng == "P":
                nc.gpsimd.tensor_scalar(h, p, btiles[k], 0.0, op0=add, op1=amax)
            else:
                nc.vector.tensor_scalar(h, p, btiles[k], 0.0, op0=add, op1=amax)

        def stt(eng, t_out, p, k, res):
            # t = (p + b_k) + res
            e = nc.gpsimd if eng == "P" else nc.vector
            e.scalar_tensor_tensor(t_out, p, btiles[k], res, op0=add, op1=add)

        def relu_plain(eng, h, t_in):
            # h = max(t, 0)
            if eng == "A":
                nc.scalar.activation(h, t_in, mybir.ActivationFunctionType.Relu)
            elif eng == "P":
                nc.gpsimd.tensor_scalar_max(h, t_in, 0.0)
            else:
                nc.vector.tensor_scalar_max(h, t_in, 0.0)

        # per-macro pipeline state + per-window (8 macros) head psum
        st = [dict() for _ in range(n_macros)]
        win = {}

        # fine stages: e=0 x-DMA; per layer k=0..10: mm=3k+1, evac1=3k+2,
        # evac2=3k+3; head mm e=34; head evac e=35; y DMA e=36.
        E_HEADMM = 34
        E_HEADEV = 35
        E_YDMA = 36
        N_STAGES = 37

        def stage_fine(t, e):
            m = st[t]
            if e == 0:
                if t % xpair != 0:
                    return
                xt = xpool.tile([32, xpair * ncols], _DT_F16, tag="x")
                tp = t // xpair
                nc.sync.dma_start(xt[:], x_d[32 * tp : 32 * tp + 32, :])
                for j in range(xpair):
                    st[t + j]["x"] = xt[:, j * ncols : (j + 1) * ncols]
                return
            if e == E_HEADMM:
                w = t // HW
                s = t % HW
                q, v = divmod(s, 2)
                if s == 0:
                    hp_new = hppool.tile([128, ncols], _DT_F32, tag="hp")
                    win[w] = hp_new
                hp = win[w]
                h = m.pop("h10")
                for c in CB:
                    nc.tensor.matmul(
                        hp[32 * q : 32 * q + 32, c],
                        wot[:, 32 * v : 32 * v + 32],
                        h[:, c],
                        start=(v == 0),
                        stop=(v == 1),
                        tile_position=(0, 32 * q),
                    )
                return
            if e == E_HEADEV:
                if t % HW != HW - 1 or heade == "DMA":
                    return
                hp = win.pop(t // HW)
                yt = ypool.tile([128, ncols], _DT_F16, tag="y")
                if heade == "A":
                    nc.scalar.copy(yt[:], hp[:])
                else:
                    nc.vector.tensor_copy(yt[:], hp[:])
                m["y"] = yt
                return
            if e == E_YDMA:
                if t % HW != HW - 1:
                    return
                w = t // HW
                if heade == "DMA":
                    hp = win.pop(w)
                    nc.sync.dma_start(y_d[128 * w : 128 * w + 128, :], hp[:])
                else:
                    yt = m.pop("y")
                    nc.sync.dma_start(y_d[128 * w : 128 * w + 128, :], yt[:])
                return
            k, ph = divmod(e - 1, 3)
            if k > 10:
                return
            is_b = k > 0 and k % 2 == 0
            on_pe = is_b and resid[k] == "pe"
            if ph == 0:  # matmul of layer k
                p = ppool.tile([128, ncols], _DT_F32, tag="p")
                if k == 0:
                    xt = m.pop("x")
                    for c in CB:
                        nc.tensor.matmul(p[:, c], w1t[:], xt[:, c], start=True, stop=True)
                elif on_pe:
                    h = m[f"h{k - 1}"]
                    res = m[f"h{k - 2}"]
                    # Alternate accumulation order per macro so paired
                    # emission runs Wb,id | id,Wb — adjacent id matmuls share
                    # one ldweights of the identity stationary.
                    for c in CB:
                        if t % 2 == 0:
                            nc.tensor.matmul(
                                p[:, c], wabt[k - 1], h[:, c], start=True, stop=False
                            )
                            nc.tensor.matmul(
                                p[:, c], idt[:], res[:, c], start=False, stop=True
                            )
                        else:
                            nc.tensor.matmul(
                                p[:, c], idt[:], res[:, c], start=True, stop=False
                            )
                            nc.tensor.matmul(
                                p[:, c], wabt[k - 1], h[:, c], start=False, stop=True
                            )
                else:
                    h = m[f"h{k - 1}"]
                    for c in CB:
                        nc.tensor.matmul(
                            p[:, c], wabt[k - 1], h[:, c], start=True, stop=True
                        )
                m["p"] = p
                return
            if ph == 1:  # evac1: relu+bias (fc1/A/B-on-pe) or stt (B-stt)
                eng = _pick(evac1, k, t)
                if not is_b or on_pe:
                    h = hpool.tile([128, ncols], _DT_F16, tag="h")
                    relu_bias(eng, h[:], m.pop("p"), k)
                    m[f"h{k}"] = h
                else:
                    tt = tpool.tile([128, ncols], _DT_F16, tag="t")
                    res = m[f"h{k - 2}"]
                    stt(eng, tt[:], m.pop("p"), k, res[:])
                    m["t"] = tt
                return
            # ph == 2: relu2 for B layers with engine residual
            if not is_b or on_pe:
                return
            eng = _pick(relu2, k, t)
            h = hpool.tile([128, ncols], _DT_F16, tag="h")
            relu_plain(eng, h[:], m.pop("t")[:])
            m[f"h{k}"] = h

        # pair>1 emits each fine stage for `pair` consecutive macros
        # back-to-back, so same-stationary matmuls run adjacently on the PE
        # (ldweights amortization).
        events = sorted(
            (((t // pair) * skew + (e if e else -xlead), t, e)
             for t in range(n_macros) for e in range(N_STAGES)),
            key=(lambda ev: (ev[0], ev[1])) if pair == 1 else
                (lambda ev: (ev[0], ev[2], ev[1])),
        )

        def emit_all():
            for rep in range(repeat):
                st[:] = [dict() for _ in range(n_macros)]
                win.clear()
                for _, t, e in events:
                    stage_fine(t, e)

        if hw_loop:
            with tc.For_i(0, hw_loop, 1):
                emit_all()
        else:
            emit_all()

    nc.compile()
    return nc


class TileCtx:
    """TileContext + ExitStack in one `with`."""

    def __init__(self, nc):
        self.nc = nc

    def __enter__(self):
        self._es = ExitStack()
        self._tc = self._es.enter_context(tile.TileContext(self.nc))
        return self._tc, self._es

    def __exit__(self, *exc):
        return self._es.__exit__(*exc)


_CACHED_NC = None


def kernel(x, W1, b1, Wh, bh, Wo, bo):
    global _CACHED_NC
    x = np.asarray(x, dtype=np.float32)
    W1 = np.asarray(W1, dtype=np.float32)
    b1 = np.asarray(b1, dtype=np.float32)
    Wh = np.asarray(Wh, dtype=np.float32)
    bh = np.asarray(bh, dtype=np.float32)
    Wo = np.asarray(Wo, dtype=np.float32)
    bo = np.asarray(bo, dtype=np.float32)

    w1s, wab, wos, bvecs, ids = _prep_weights(W1, b1, Wh, bh, Wo, bo)
    xprep = _prep_x(x, ncols=BEST["ncols"], xpair=BEST.get("xpair", 1))

    if _CACHED_NC is None:
        _CACHED_NC = build_module(num_devices=N_CORES, **BEST)
    nc = _CACHED_NC

    in_maps = [
        {
            "xprep": np.ascontiguousarray(xprep[c]),
            "w1s": w1s,
            "wab": wab,
            "wos": wos,
            "bvecs": bvecs,
            "ids": ids,
        }
        for c in range(N_CORES)
    ]
    res = run_bass_kernel_spmd(nc, in_maps, core_ids=list(range(N_CORES)))
    yparts = [res.results[c]["yprep"] for c in range(N_CORES)]
    return _post_y(yparts, bo, ncols=BEST["ncols"])

